# revision 20
# baseline (speedup 1.0000x reference)
"""BoW extractor (VQ codebook softmax + border-cropped mean pool) on 8 Trainium2 cores.

Data-parallel over the batch dim: each of the 8 NeuronCores handles 16 images.
Per core, tokens are flattened to [3136, 768] (padded to 3200 = 25 tiles of 128)
and processed tile-by-tile:
  logits = 30 * (x @ emb.T) / ||x||   (fp32r matmuls, C contracted in 6 chunks)
  codes  = exp(logits) / row_sum      (ACT exp with per-partition scale + fused
                                       row-sums, DVE reciprocal + scale)
  bow    = L1norm(mean of kept codes) (PE matmul against a precomputed selector
                                       W, PSUM results DVE-accumulated in SBUF)
The first 3 tiles are processed codebook-block-outer so the PE consumes the
codebook exactly in DMA-arrival order (keeps the PE dense at startup, which
also keeps the HAM clock-gate open).
"""
import sys

sys.path.insert(0, "/opt/trn_rl_repo")

import numpy as np

N_CORES = 8
N_IMG = 128
C = 768
K = 4096
L = 196  # tokens per image after dropping CLS
IMG_PER_CORE = N_IMG // N_CORES  # 16
T_TOK = IMG_PER_CORE * L  # 3136
NT = 25  # 128-token tiles per core
T_PAD = NT * 128  # 3200
GRID = 14
SKIP = 2
INV_DELTA = 30.0  # 15.0 / 0.5
NORMALIZE_EPS = 1e-5
NB = 5  # tiles per norm (sqrt) batch
NPRE = 4  # token tiles prefetched ahead of the codebook load

PROFILE = False
LAST_EXEC_NS = None

_PROG = None
_HOST_CONST = None


def _build_program():
    import concourse.bacc as bacc
    import concourse.tile as tile
    from concourse import mybir

    f32 = mybir.dt.float32
    f32r = mybir.dt.float32r
    bf16 = mybir.dt.bfloat16
    AF = mybir.ActivationFunctionType
    AX = mybir.AxisListType

    nc = bacc.Bacc("TRN2", target_bir_lowering=False, debug=False,
                   num_devices=N_CORES)
    xT_d = nc.dram_tensor("xT", [NT, 128, 6, 128], f32r, kind="ExternalInput")
    xn_d = nc.dram_tensor("xnat", [NT, 128, C], f32, kind="ExternalInput")
    emb_d = nc.dram_tensor("embT", [8, 128, 6, 512], f32r,
                           kind="ExternalInput")
    w_d = nc.dram_tensor("W", [128, NT, 16], f32r, kind="ExternalInput")
    codes_d = nc.dram_tensor("codes", [NT, 128, 8, 512], f32r,
                             kind="ExternalOutput")
    bow_d = nc.dram_tensor("bow", [16, 8, 512], f32, kind="ExternalOutput")

    with tile.TileContext(nc) as tc:
        with (
            tc.tile_pool(name="const", bufs=1) as constp,
            tc.tile_pool(name="xT", bufs=NPRE, space="SBUF") as xTp,
            tc.tile_pool(name="xn", bufs=2) as xnp_,
            tc.tile_pool(name="expp", bufs=3) as expp,
            tc.tile_pool(name="small", bufs=4) as smallp,
            tc.tile_pool(name="ps", bufs=6, space="PSUM") as psp,
            tc.tile_pool(name="psbow", bufs=2, space="PSUM") as psbowp,
        ):
            # --- PE warmup: dependency-free dummy matmuls run during the
            # initial DMAs so the HAM clock-gate opens before real work.
            warm_sb = constp.tile([128, 128], bf16)
            nc.vector.memset(warm_sb[:], 0.0)
            warm_ps = psp.tile([128, 512], f32, name="warm_ps", tag="ps")
            for i in range(60):
                nc.tensor.matmul(warm_ps[:, :128], warm_sb[:], warm_sb[:],
                                 start=True, stop=True)

            # --- prefetch the first NPRE token tiles ahead of the codebook
            pre_xT = []
            for t in range(NPRE):
                a = xTp.tile([128, 6, 128], f32r, name=f"xTpre{t}",
                             tag="xT_t")
                nc.sync.dma_start(out=a[:], in_=xT_d.ap()[t])
                pre_xT.append(a)

            w_sb = constp.tile([128, NT, 16], f32r)
            nc.scalar.dma_start(out=w_sb[:], in_=w_d.ap())

            bow_acc = constp.tile([16, 8, 512], f32)
            nc.gpsimd.memset(bow_acc[:], 0.0)

            # --- norm pipeline, one NB-tile batch ahead of the main loop:
            # squares accumulate ||x||^2, one batched ACT sqrt per NB tiles
            # (2 table-set reloads per batch instead of per tile)
            n2_all = constp.tile([128, NT], f32)
            sinv_all = constp.tile([128, NT], f32)
            sq_scr = constp.tile([128, C], f32)  # discarded square output

            def norm_batch(g):
                lo, hi = NB * g, min(NB * (g + 1), NT)
                for u in range(lo, hi):
                    xn_u = xnp_.tile([128, C], f32, name=f"xn{u}", tag="xn")
                    nc.sync.dma_start(out=xn_u[:], in_=xn_d.ap()[u])
                    nc.scalar.activation(sq_scr[:], xn_u[:], AF.Square,
                                         accum_out=n2_all[:, u:u + 1])
                nc.vector.tensor_scalar_max(n2_all[:, lo:hi],
                                            n2_all[:, lo:hi],
                                            NORMALIZE_EPS * NORMALIZE_EPS)
                nrm = smallp.tile([128, NB], f32, name=f"nrm{g}", tag="nrm")
                nc.scalar.activation(nrm[:, :hi - lo], n2_all[:, lo:hi],
                                     AF.Sqrt,
                                     scale=1.0 / (INV_DELTA * INV_DELTA))
                nc.vector.reciprocal(sinv_all[:, lo:hi], nrm[:, :hi - lo])

            norm_batch(0)

            # --- codebook load split across both HWDGE rings, interleaved
            # so blocks land roughly in consumption order
            emb_sb = constp.tile([128, 8, 6, 512], f32r)
            for j in range(8):
                eng = nc.scalar if j % 2 == 0 else nc.sync
                eng.dma_start(out=emb_sb[:, j], in_=emb_d.ap()[j])

            def get_xT(t):
                if t < NPRE:
                    return pre_xT[t]
                xT_t = xTp.tile([128, 6, 128], f32r, name=f"xT{t}",
                                tag="xT_t")
                nc.sync.dma_start(out=xT_t[:], in_=xT_d.ap()[t])
                return xT_t

            def new_exp_sums(t):
                exp_t = expp.tile([128, 8, 512], f32r, name=f"exp{t}",
                                  tag="exp_t")
                sums = smallp.tile([128, 8], f32, name=f"sums{t}",
                                   tag="sums")
                return exp_t, sums

            def do_block(t, j, xT_t, exp_t, sums):
                ps = psp.tile([128, 512], f32, name=f"ps{t}_{j}", tag="ps")
                for c in range(6):
                    nc.tensor.matmul(ps[:], xT_t[:, c, :],
                                     emb_sb[:, j, c, :],
                                     start=(c == 0), stop=(c == 5))
                nc.scalar.activation(exp_t[:, j, :], ps[:], AF.Exp,
                                     scale=sinv_all[:, t:t + 1],
                                     accum_out=sums[:, j:j + 1])

            def do_post(t, exp_t, sums):
                denom = smallp.tile([128, 1], f32, name=f"dn{t}",
                                    tag="denom")
                nc.vector.reduce_sum(denom[:], sums[:], axis=AX.X)
                r = smallp.tile([128, 1], f32, name=f"r{t}", tag="r")
                nc.vector.reciprocal(r[:], denom[:])
                for j in range(8):
                    nc.vector.tensor_scalar_mul(exp_t[:, j, :],
                                                exp_t[:, j, :], r[:])
                    bow_tmp = psbowp.tile([16, 512], f32,
                                          name=f"bt{t}_{j}", tag="bt")
                    nc.tensor.matmul(bow_tmp[:], w_sb[:, t, :],
                                     exp_t[:, j, :], start=True, stop=True)
                    nc.vector.tensor_add(bow_acc[:, j, :], bow_acc[:, j, :],
                                         bow_tmp[:])
                    if j == 3:
                        nc.gpsimd.dma_start(out=codes_d.ap()[t][:, 0:4, :],
                                            in_=exp_t[:, 0:4, :])
                nc.gpsimd.dma_start(out=codes_d.ap()[t][:, 4:8, :],
                                    in_=exp_t[:, 4:8, :])

            # --- startup: tiles 0-2 block-outer (follows emb DMA arrival)
            NSTART = 3
            start_bufs = [new_exp_sums(t) for t in range(NSTART)]
            for j in range(8):
                for t in range(NSTART):
                    do_block(t, j, pre_xT[t], *start_bufs[t])
            for t in range(NSTART):
                do_post(t, *start_bufs[t])

            # --- steady state
            for t in range(NSTART, NT):
                # emit norm batch g two tiles before tile 5g needs it
                if (t + 2) % NB == 0 and (t + 2) // NB <= (NT - 1) // NB:
                    norm_batch((t + 2) // NB)
                xT_t = get_xT(t)
                exp_t, sums = new_exp_sums(t)
                for j in range(8):
                    do_block(t, j, xT_t, exp_t, sums)
                do_post(t, exp_t, sums)

            # --- L1-normalize bow per image (rows are images)
            ssum = smallp.tile([16, 1], f32)
            nc.vector.reduce_sum(ssum[:], bow_acc[:], axis=AX.XY)
            nc.vector.tensor_scalar_max(ssum[:], ssum[:], NORMALIZE_EPS)
            rimg = smallp.tile([16, 1], f32)
            nc.vector.reciprocal(rimg[:], ssum[:])
            nc.vector.tensor_scalar_mul(bow_acc[:], bow_acc[:], rimg[:])
            nc.gpsimd.dma_start(out=bow_d.ap(), in_=bow_acc[:])

    nc.compile()
    return nc


def _host_constants():
    global _HOST_CONST
    if _HOST_CONST is not None:
        return _HOST_CONST
    # kept-token mask on the 14x14 grid (drop SKIP-wide border)
    l_idx = np.arange(L)
    row, col = l_idx // GRID, l_idx % GRID
    kept = ((row >= SKIP) & (row < GRID - SKIP) &
            (col >= SKIP) & (col < GRID - SKIP))
    n_keep = int(kept.sum())  # 100
    w_full = np.zeros((T_PAD, IMG_PER_CORE), np.float32)
    for i in range(IMG_PER_CORE):
        w_full[i * L:(i + 1) * L, i] = kept / float(n_keep)
    w3 = np.ascontiguousarray(
        w_full.reshape(NT, 128, IMG_PER_CORE).transpose(1, 0, 2))
    _HOST_CONST = w3
    return _HOST_CONST


def _get_program():
    global _PROG
    if _PROG is None:
        _PROG = _build_program()
    return _PROG


def kernel(x, embedding):
    global LAST_EXEC_NS
    from concourse.bass_utils import run_bass_kernel_spmd

    x = np.asarray(x, dtype=np.float32)
    embedding = np.asarray(embedding, dtype=np.float32)
    nc = _get_program()
    w3 = _host_constants()

    embT = np.ascontiguousarray(
        embedding.T.reshape(6, 128, 8, 512).transpose(2, 1, 0, 3))

    in_maps = []
    for core in range(N_CORES):
        xc = x[core * IMG_PER_CORE:(core + 1) * IMG_PER_CORE, 1:, :]
        xp = np.zeros((T_PAD, C), np.float32)
        xp[:T_TOK] = xc.reshape(T_TOK, C)
        xp[T_TOK:, 0] = 1.0  # pad tokens: unit norm, zero pool weight
        in_maps.append({
            "xT": np.ascontiguousarray(
                xp.reshape(NT, 128, 6, 128).transpose(0, 3, 2, 1)),
            "xnat": xp.reshape(NT, 128, C),
            "embT": embT,
            "W": w3,
        })

    res = run_bass_kernel_spmd(nc, in_maps, core_ids=list(range(N_CORES)),
                               trace=PROFILE)
    LAST_EXEC_NS = res.exec_time_ns

    bow = np.empty((N_IMG, K), np.float32)
    codes = np.empty((N_IMG, L, K), np.float32)
    for core in range(N_CORES):
        sl = slice(core * IMG_PER_CORE, (core + 1) * IMG_PER_CORE)
        codes[sl] = (res.results[core]["codes"]
                     .reshape(T_PAD, K)[:T_TOK]
                     .reshape(IMG_PER_CORE, L, K))
        bow[sl] = res.results[core]["bow"].reshape(IMG_PER_CORE, K)
    return bow, codes


# revision 21
# speedup vs baseline: 1.0740x; 1.0740x over previous
"""BoW extractor (VQ codebook softmax + border-cropped mean pool) on 8 Trainium2 cores.

Data-parallel over the batch dim: each of the 8 NeuronCores handles 16 images.
Per core, tokens are flattened to [3136, 768] (padded to 3200 = 25 tiles of 128)
and processed tile-by-tile:
  logits = 30 * (x @ emb.T) / ||x||   (fp32r matmuls, C contracted in 6 chunks)
  codes  = exp(logits) / row_sum      (ACT exp with per-partition scale + fused
                                       row-sums, DVE reciprocal + scale)
  bow    = L1norm(mean of kept codes) (PE matmul against a precomputed selector
                                       W, PSUM results DVE-accumulated in SBUF)
The first 3 tiles are processed codebook-block-outer so the PE consumes the
codebook exactly in DMA-arrival order (keeps the PE dense at startup, which
also keeps the HAM clock-gate open).
"""
import sys

sys.path.insert(0, "/opt/trn_rl_repo")

import numpy as np

N_CORES = 8
N_IMG = 128
C = 768
K = 4096
L = 196  # tokens per image after dropping CLS
IMG_PER_CORE = N_IMG // N_CORES  # 16
T_TOK = IMG_PER_CORE * L  # 3136
NT = 25  # 128-token tiles per core
T_PAD = NT * 128  # 3200
GRID = 14
SKIP = 2
INV_DELTA = 30.0  # 15.0 / 0.5
NORMALIZE_EPS = 1e-5
NB = 5  # tiles per norm (sqrt) batch
NPRE = 4  # token tiles prefetched ahead of the codebook load

PROFILE = False
LAST_EXEC_NS = None

_PROG = None
_HOST_CONST = None


def _build_program():
    import concourse.bacc as bacc
    import concourse.tile as tile
    from concourse import mybir

    f32 = mybir.dt.float32
    f32r = mybir.dt.float32r
    bf16 = mybir.dt.bfloat16
    AF = mybir.ActivationFunctionType
    AX = mybir.AxisListType

    nc = bacc.Bacc("TRN2", target_bir_lowering=False, debug=False,
                   num_devices=N_CORES)
    xT_d = nc.dram_tensor("xT", [NT, 128, 6, 128], bf16, kind="ExternalInput")
    xn_d = nc.dram_tensor("xnat", [NT, 128, C], f32, kind="ExternalInput")
    emb_d = nc.dram_tensor("embT", [8, 128, 6, 512], bf16,
                           kind="ExternalInput")
    w_d = nc.dram_tensor("W", [128, NT, 16], f32r, kind="ExternalInput")
    codes_d = nc.dram_tensor("codes", [NT, 128, 8, 512], f32r,
                             kind="ExternalOutput")
    bow_d = nc.dram_tensor("bow", [16, 8, 512], f32, kind="ExternalOutput")

    with tile.TileContext(nc) as tc:
        with (
            tc.tile_pool(name="const", bufs=1) as constp,
            tc.tile_pool(name="xT", bufs=NPRE, space="SBUF") as xTp,
            tc.tile_pool(name="xn", bufs=2) as xnp_,
            tc.tile_pool(name="expp", bufs=3) as expp,
            tc.tile_pool(name="small", bufs=4) as smallp,
            tc.tile_pool(name="ps", bufs=6, space="PSUM") as psp,
            tc.tile_pool(name="psbow", bufs=2, space="PSUM") as psbowp,
        ):
            # --- PE warmup: dependency-free dummy matmuls run during the
            # initial DMAs so the HAM clock-gate opens before real work.
            warm_sb = constp.tile([128, 128], bf16)
            nc.vector.memset(warm_sb[:], 0.0)
            warm_ps = psp.tile([128, 512], f32, name="warm_ps", tag="ps")
            for i in range(60):
                nc.tensor.matmul(warm_ps[:, :128], warm_sb[:], warm_sb[:],
                                 start=True, stop=True)

            # --- prefetch the first NPRE token tiles ahead of the codebook
            pre_xT = []
            for t in range(NPRE):
                a = xTp.tile([128, 6, 128], bf16, name=f"xTpre{t}",
                             tag="xT_t")
                nc.sync.dma_start(out=a[:], in_=xT_d.ap()[t])
                pre_xT.append(a)

            bow_acc = constp.tile([16, 8, 512], f32)
            nc.gpsimd.memset(bow_acc[:], 0.0)

            # --- norm pipeline, one NB-tile batch ahead of the main loop:
            # squares accumulate ||x||^2, one batched ACT sqrt per NB tiles
            # (2 table-set reloads per batch instead of per tile)
            n2_all = constp.tile([128, NT], f32)
            sinv_all = constp.tile([128, NT], f32)
            sq_scr = constp.tile([128, C], f32)  # discarded square output

            def norm_batch(g):
                lo, hi = NB * g, min(NB * (g + 1), NT)
                for u in range(lo, hi):
                    xn_u = xnp_.tile([128, C], f32, name=f"xn{u}", tag="xn")
                    nc.sync.dma_start(out=xn_u[:], in_=xn_d.ap()[u])
                    nc.vector.scalar_tensor_tensor(
                        sq_scr[:], xn_u[:], 1.0, xn_u[:],
                        mybir.AluOpType.mult, mybir.AluOpType.mult,
                        accum_out=n2_all[:, u:u + 1])
                nc.vector.tensor_scalar_max(n2_all[:, lo:hi],
                                            n2_all[:, lo:hi],
                                            NORMALIZE_EPS * NORMALIZE_EPS)
                nrm = smallp.tile([128, NB], f32, name=f"nrm{g}", tag="nrm")
                nc.scalar.activation(nrm[:, :hi - lo], n2_all[:, lo:hi],
                                     AF.Sqrt,
                                     scale=1.0 / (INV_DELTA * INV_DELTA))
                nc.vector.reciprocal(sinv_all[:, lo:hi], nrm[:, :hi - lo])

            norm_batch(0)

            # --- codebook load split across both HWDGE rings, interleaved
            # so blocks land roughly in consumption order
            emb_sb = constp.tile([128, 8, 6, 512], bf16)
            for j in range(8):
                eng = nc.scalar if j % 2 == 0 else nc.sync
                eng.dma_start(out=emb_sb[:, j], in_=emb_d.ap()[j])
            w_sb = constp.tile([128, NT, 16], f32r)
            nc.scalar.dma_start(out=w_sb[:], in_=w_d.ap())

            def get_xT(t):
                if t < NPRE:
                    return pre_xT[t]
                xT_t = xTp.tile([128, 6, 128], bf16, name=f"xT{t}",
                                tag="xT_t")
                nc.sync.dma_start(out=xT_t[:], in_=xT_d.ap()[t])
                return xT_t

            def new_exp_sums(t):
                exp_t = expp.tile([128, 8, 512], f32r, name=f"exp{t}",
                                  tag="exp_t")
                sums = smallp.tile([128, 8], f32, name=f"sums{t}",
                                   tag="sums")
                return exp_t, sums

            def do_block(t, j, xT_t, exp_t, sums):
                ps = psp.tile([128, 512], f32, name=f"ps{t}_{j}", tag="ps")
                for c in range(6):
                    nc.tensor.matmul(ps[:], xT_t[:, c, :],
                                     emb_sb[:, j, c, :],
                                     start=(c == 0), stop=(c == 5))
                nc.scalar.activation(exp_t[:, j, :], ps[:], AF.Exp,
                                     scale=sinv_all[:, t:t + 1],
                                     accum_out=sums[:, j:j + 1])

            def do_post(t, exp_t, sums):
                denom = smallp.tile([128, 1], f32, name=f"dn{t}",
                                    tag="denom")
                nc.vector.reduce_sum(denom[:], sums[:], axis=AX.X)
                r = smallp.tile([128, 1], f32, name=f"r{t}", tag="r")
                nc.vector.reciprocal(r[:], denom[:])
                for j in range(8):
                    nc.vector.tensor_scalar_mul(exp_t[:, j, :],
                                                exp_t[:, j, :], r[:])
                    bow_tmp = psbowp.tile([16, 512], f32,
                                          name=f"bt{t}_{j}", tag="bt")
                    nc.tensor.matmul(bow_tmp[:], w_sb[:, t, :],
                                     exp_t[:, j, :], start=True, stop=True)
                    nc.vector.tensor_add(bow_acc[:, j, :], bow_acc[:, j, :],
                                         bow_tmp[:])
                    if j == 3:
                        nc.gpsimd.dma_start(out=codes_d.ap()[t][:, 0:4, :],
                                            in_=exp_t[:, 0:4, :])
                nc.gpsimd.dma_start(out=codes_d.ap()[t][:, 4:8, :],
                                    in_=exp_t[:, 4:8, :])

            # --- startup: tiles 0-2 block-outer (follows emb DMA arrival)
            NSTART = 3
            start_bufs = [new_exp_sums(t) for t in range(NSTART)]
            for j in range(8):
                for t in range(NSTART):
                    do_block(t, j, pre_xT[t], *start_bufs[t])
            for t in range(NSTART):
                do_post(t, *start_bufs[t])

            # --- steady state
            for t in range(NSTART, NT):
                # emit norm batch g two tiles before tile 5g needs it
                if (t + 2) % NB == 0 and (t + 2) // NB <= (NT - 1) // NB:
                    norm_batch((t + 2) // NB)
                xT_t = get_xT(t)
                exp_t, sums = new_exp_sums(t)
                for j in range(8):
                    do_block(t, j, xT_t, exp_t, sums)
                do_post(t, exp_t, sums)

            # --- L1-normalize bow per image (rows are images)
            ssum = smallp.tile([16, 1], f32)
            nc.vector.reduce_sum(ssum[:], bow_acc[:], axis=AX.XY)
            nc.vector.tensor_scalar_max(ssum[:], ssum[:], NORMALIZE_EPS)
            rimg = smallp.tile([16, 1], f32)
            nc.vector.reciprocal(rimg[:], ssum[:])
            nc.vector.tensor_scalar_mul(bow_acc[:], bow_acc[:], rimg[:])
            nc.gpsimd.dma_start(out=bow_d.ap(), in_=bow_acc[:])

    nc.compile()
    return nc


def _host_constants():
    global _HOST_CONST
    if _HOST_CONST is not None:
        return _HOST_CONST
    # kept-token mask on the 14x14 grid (drop SKIP-wide border)
    l_idx = np.arange(L)
    row, col = l_idx // GRID, l_idx % GRID
    kept = ((row >= SKIP) & (row < GRID - SKIP) &
            (col >= SKIP) & (col < GRID - SKIP))
    n_keep = int(kept.sum())  # 100
    w_full = np.zeros((T_PAD, IMG_PER_CORE), np.float32)
    for i in range(IMG_PER_CORE):
        w_full[i * L:(i + 1) * L, i] = kept / float(n_keep)
    w3 = np.ascontiguousarray(
        w_full.reshape(NT, 128, IMG_PER_CORE).transpose(1, 0, 2))
    _HOST_CONST = w3
    return _HOST_CONST


def _get_program():
    global _PROG
    if _PROG is None:
        _PROG = _build_program()
    return _PROG


def kernel(x, embedding):
    global LAST_EXEC_NS
    from concourse.bass_utils import run_bass_kernel_spmd

    x = np.asarray(x, dtype=np.float32)
    embedding = np.asarray(embedding, dtype=np.float32)
    nc = _get_program()
    w3 = _host_constants()

    import ml_dtypes
    bf16 = ml_dtypes.bfloat16
    embT = np.ascontiguousarray(
        embedding.T.reshape(6, 128, 8, 512).transpose(2, 1, 0, 3)
        .astype(bf16))

    in_maps = []
    for core in range(N_CORES):
        xc = x[core * IMG_PER_CORE:(core + 1) * IMG_PER_CORE, 1:, :]
        xp = np.zeros((T_PAD, C), np.float32)
        xp[:T_TOK] = xc.reshape(T_TOK, C)
        xp[T_TOK:, 0] = 1.0  # pad tokens: unit norm, zero pool weight
        in_maps.append({
            "xT": np.ascontiguousarray(
                xp.reshape(NT, 128, 6, 128).transpose(0, 3, 2, 1)
                .astype(bf16)),
            "xnat": xp.reshape(NT, 128, C),
            "embT": embT,
            "W": w3,
        })

    res = run_bass_kernel_spmd(nc, in_maps, core_ids=list(range(N_CORES)),
                               trace=PROFILE)
    LAST_EXEC_NS = res.exec_time_ns

    bow = np.empty((N_IMG, K), np.float32)
    codes = np.empty((N_IMG, L, K), np.float32)
    for core in range(N_CORES):
        sl = slice(core * IMG_PER_CORE, (core + 1) * IMG_PER_CORE)
        codes[sl] = (res.results[core]["codes"]
                     .reshape(T_PAD, K)[:T_TOK]
                     .reshape(IMG_PER_CORE, L, K))
        bow[sl] = res.results[core]["bow"].reshape(IMG_PER_CORE, K)
    return bow, codes
